# revision 2
# baseline (speedup 1.0000x reference)
"""DRNL filterbank Trainium2 kernel, v3.

Banded-Toeplitz formulation (see kernel.py). Engine-balanced pipeline:
  - ME + LIN FIRs: bf16 matmuls.
  - BEF FIR: fp8e4m3 DoubleRow band pairs (k-tile 1 reads a one-column-
    shifted second copy of the fp8 signal at a large stride — the PE
    rejects small k-tile strides).
  - AFT FIR: bf16 single-band matmuls on the bf16 broken-stick output
    (pairing AFT would need a per-chunk shifted copy of w, which choked
    the slow gpsimd engine in v2).
  - Broken-stick: w = clip(a*v, +-b|v|^0.25). The threshold c=b|v|^0.25
    is computed two ways, alternated per chunk to balance engines:
      "sqrt": c = sqrt(sqrt(b^4|v|))          (3 scalar-engine acts)
      "int":  c ~= bitcast((bits(v)&0x7fffffff)>>2 + K_f)  (1 vector
              tensor_scalar + 1 scalar Identity-with-int-bias act);
              K_f host-tuned per channel, ~3% error on c — harmless,
              the NL path is ~500x below the linear path.
  - Output copies (psum -> bf16) alternate scalar/vector per chunk.
  - lag-2 software pipeline; heavy/light slot interleave; LIN issued
    before BEF so the PE fills while fp8 signal copies are produced.
Sharding: channels across 8 cores, slot-structured SPMD (as v1).
"""
import numpy as np
import ml_dtypes

P = 128
B, T, F = 8, 20000, 50
R = (T + P - 1) // P
N_CORES = 8
N_SLOTS = 7
TRUNC_LIN = 6e-4
TRUNC_NL = 2.5e-2
# slots whose AFT conv runs as fp8 DoubleRow band pairs (their channels sit
# well below the global absmax, and the whole channel is pre-scaled by a
# power of two that the host decode divides back out)
AFT_FP8_SLOTS = ()
IR_LEN = 4096
BME = 5
PADX = 4

_CACHE = {}

BF16 = ml_dtypes.bfloat16
FP8 = ml_dtypes.float8_e4m3fn

# chunk emission order: interleave heavy and light slots. First group leads
# with a tiny slot (cheap pipeline fill), last group ends on the lightest
# (short drain).
CHUNK_SLOTS = [4, 0, 6, 1, 5, 2, 3]
_GROUP_ORDERS = {
    (0, 0): [4, 0, 6, 1, 5, 2, 3],
    (0, 1): [0, 6, 1, 5, 2, 4, 3],
    (1, 0): [0, 6, 1, 5, 2, 4, 3],
    (1, 1): [0, 6, 1, 5, 2, 3, 4],
}
CHUNKS = [(s, h, ci) for (h, ci) in [(0, 0), (0, 1), (1, 0), (1, 1)]
          for s in _GROUP_ORDERS[(h, ci)]]
# per-chunk c-computation mode: 18 "int" / 10 "sqrt" balances scalar vs vector
C_MODE = ["int" if (j * 18) % 28 < 18 else "sqrt" for j in range(len(CHUNKS))]
# psum->bf16 output copy engine per chunk
COPY_ENG = ["scalar"] * len(CHUNKS)
BEF_LAG = 1   # BEF_j issues after LIN_{j+BEF_LAG}
AFT_LAG = 3   # AFT_j issues in iteration j+AFT_LAG


def _lfilter_vec(x, b, a):
    b0, b1, b2 = b[:, 0], b[:, 1], b[:, 2]
    a1, a2 = a[:, 1], a[:, 2]
    y = np.zeros_like(x)
    z1 = np.zeros(x.shape[0])
    z2 = np.zeros(x.shape[0])
    for t in range(x.shape[-1]):
        xt = x[:, t]
        yt = b0 * xt + z1
        z1 = b1 * xt - a1 * yt + z2
        z2 = b2 * xt - a2 * yt
        y[:, t] = yt
    return y


def _cascade_ir(b, a, n, times):
    h = np.zeros((b.shape[0], n))
    h[:, 0] = 1.0
    for _ in range(times):
        h = _lfilter_vec(h, b, a)
    return h


def _trunc(h, tol):
    m = np.abs(h).max()
    idx = np.nonzero(np.abs(h) > tol * m)[0]
    return h[: int(idx[-1]) + 1] if len(idx) else h[:1]


def _nb(h):
    return (len(h) + P - 2) // P + 1


def _toeplitz_band(h, d):
    k = np.arange(P)[:, None]
    i = np.arange(P)[None, :]
    idx = P * d + i - k
    ok = (idx >= 0) & (idx < len(h))
    return np.where(ok, np.asarray(h, np.float64)[np.clip(idx, 0, len(h) - 1)], 0.0)


def _bands_cat(h, nb, dtype):
    W = np.concatenate([_toeplitz_band(h, d) for d in range(nb)], axis=1)
    return np.ascontiguousarray(W).astype(dtype)


def _pairs_cat(h, nbp, dtype):
    mats = []
    for p in range(nbp):
        mats.append(_toeplitz_band(h, 2 * p))
        mats.append(_toeplitz_band(h, 2 * p + 1))
    W = np.concatenate(mats, axis=1)
    return np.ascontiguousarray(W).astype(dtype)


def _tune_k(b):
    """Best int32 bias K: bitcast((bits(x)>>2)+K) ~= b*x**0.25 over x range."""
    x = np.float32(np.logspace(-6, 1.5, 4000))
    t1 = (x.view(np.int32) & 0x7FFFFFFF) >> 2
    ref = np.float64(b) * np.float64(x) ** 0.25
    b0 = 127 << 23
    base = np.int64(np.float32(b).view(np.int32)) - b0 // 4
    best = (1e9, 0)
    for sig in np.linspace(-0.02, 0.10, 121):
        K = np.int64(base - int(sig * (1 << 23)))
        c = (t1.astype(np.int64) + K).astype(np.int32).view(np.float32)
        e = np.abs(c - ref) / ref
        best = min(best, (float(e.max()), int(K)))
    return best[1]


def _build_host(me_fir, lin_fir, nlin_fir_before, nlin_fir_after,
                lpf_lin_b, lpf_lin_a, lpf_nlin_b, lpf_nlin_a,
                lin_gain, nlin_a, nlin_b):
    ir4 = _cascade_ir(lpf_lin_b.astype(np.float64), lpf_lin_a.astype(np.float64), IR_LEN, 4)
    ir3 = _cascade_ir(lpf_nlin_b.astype(np.float64), lpf_nlin_a.astype(np.float64), IR_LEN, 3)

    scale = 10.0 ** ((93.98 - 100.0) / 20.0)
    ME = np.asarray(me_fir, np.float64) * scale

    LIN, BEF, AFT = [], [], []
    for f in range(F):
        LIN.append(_trunc(lin_gain[f] * np.convolve(np.asarray(lin_fir[f], np.float64), ir4[f]), TRUNC_LIN))
        BEF.append(_trunc(np.asarray(nlin_fir_before[f], np.float64), TRUNC_NL))
        AFT.append(_trunc(np.convolve(np.asarray(nlin_fir_after[f], np.float64), ir3[f]), TRUNC_NL))

    nbp = lambda h: (_nb(h) + 1) // 2
    cost = [_nb(LIN[f]) + nbp(BEF[f]) + _nb(AFT[f]) for f in range(F)]
    order = np.argsort(-np.asarray(cost), kind="stable")

    slot_ch = np.zeros((N_CORES, N_SLOTS), np.int64)
    for s in range(6):
        for c in range(N_CORES):
            slot_ch[c, s] = order[8 * s + c]
    for c in range(N_CORES):
        slot_ch[c, 6] = order[48 + (c % 2)]

    BL = [max(_nb(LIN[slot_ch[c, s]]) for c in range(N_CORES)) for s in range(N_SLOTS)]
    BBp = [max(nbp(BEF[slot_ch[c, s]]) for c in range(N_CORES)) for s in range(N_SLOTS)]
    # AFT band count: pairs for fp8 slots, single bands otherwise
    BA = []
    for s in range(N_SLOTS):
        if s in AFT_FP8_SLOTS:
            BA.append(max(nbp(AFT[slot_ch[c, s]]) for c in range(N_CORES)))
        else:
            BA.append(max(_nb(AFT[slot_ch[c, s]]) for c in range(N_CORES)))

    sh = max(max(BL) - 1, 2 * max(BBp) - 1,
             max((2 * BA[s] - 1) if s in AFT_FP8_SLOTS else (BA[s] - 1)
                 for s in range(N_SLOTS)))
    PADS = (sh + 1 + 1) // 2 * 2

    # per-channel power-of-two output scale: lifts fp8 AFT taps into the
    # representable range; the decode divides it back out
    gam = np.ones((N_CORES, N_SLOTS), np.float64)
    for c in range(N_CORES):
        for s in AFT_FP8_SLOTS:
            f = slot_ch[c, s]
            gam[c, s] = 2.0 ** np.floor(np.log2(64.0 / np.abs(AFT[f]).max()))

    wme = _bands_cat(ME, BME, BF16)
    wlin = [np.concatenate([_bands_cat(gam[c, s] * LIN[slot_ch[c, s]], BL[s], BF16)
                            for s in range(N_SLOTS)], axis=1)
            for c in range(N_CORES)]
    wbef = [np.concatenate([_pairs_cat(BEF[slot_ch[c, s]], BBp[s], FP8) for s in range(N_SLOTS)], axis=1)
            for c in range(N_CORES)]
    waft8, waft16 = [], []
    for c in range(N_CORES):
        p8 = [_pairs_cat(gam[c, s] * AFT[slot_ch[c, s]], BA[s], FP8)
              for s in range(N_SLOTS) if s in AFT_FP8_SLOTS]
        p16 = [_bands_cat(gam[c, s] * AFT[slot_ch[c, s]], BA[s], BF16)
               for s in range(N_SLOTS) if s not in AFT_FP8_SLOTS]
        waft8.append(np.concatenate(p8, axis=1) if p8 else np.zeros((P, 2 * P), FP8))
        waft16.append(np.concatenate(p16, axis=1) if p16 else np.zeros((P, P), BF16))

    scal = np.zeros((N_CORES, N_SLOTS * 4), np.float32)
    for c in range(N_CORES):
        for s in range(N_SLOTS):
            f = slot_ch[c, s]
            scal[c, 4 * s + 0] = nlin_a[f]
            scal[c, 4 * s + 1] = float(nlin_b[f]) ** 4
            scal[c].view(np.int32)[4 * s + 2] = _tune_k(float(nlin_b[f]))

    return {
        "slot_ch": slot_ch, "BL": BL, "BBp": BBp, "BA": BA, "PADS": PADS,
        "gam": gam,
        "wme": wme, "wlin": wlin, "wbef": wbef,
        "waft8": waft8, "waft16": waft16, "scal": scal,
    }


def _fold_x(x):
    xp = np.zeros((B, R * P), np.float32)
    xp[:, :T] = x
    xf = np.zeros((B, P, PADX + R), np.float32)
    xf[:, :, PADX:] = xp.reshape(B, R, P).transpose(0, 2, 1)
    return xf.astype(BF16)


def _build_program(meta):
    import concourse.bacc as bacc
    import concourse.bass as bass
    from concourse import mybir
    from concourse.tile import TileContext

    BL, BBp, BA, PADS = meta["BL"], meta["BBp"], meta["BA"], meta["PADS"]
    SBL, SBBp = sum(BL), sum(BBp)
    SBA8 = max(sum(BA[s] for s in range(N_SLOTS) if s in AFT_FP8_SLOTS), 1)
    SBA16 = max(sum(BA[s] for s in range(N_SLOTS) if s not in AFT_FP8_SLOTS), 1)
    SEC = PADS + R
    WW = SEC + R
    XSEC = PADX + R
    XW = XSEC + R
    f32, bf16, f8 = mybir.dt.float32, mybir.dt.bfloat16, mybir.dt.float8e4
    i32 = mybir.dt.int32
    AF = mybir.ActivationFunctionType
    ALU = mybir.AluOpType
    DR = mybir.MatmulPerfMode.DoubleRow

    nc = bacc.Bacc("TRN2", target_bir_lowering=False, debug=False, num_devices=N_CORES)
    d_xf = nc.dram_tensor("xf", [B, P, XSEC], bf16, kind="ExternalInput").ap()
    d_wme = nc.dram_tensor("wme", [P, BME * P], bf16, kind="ExternalInput").ap()
    d_wlin = nc.dram_tensor("wlin", [P, SBL * P], bf16, kind="ExternalInput").ap()
    d_wbef = nc.dram_tensor("wbef", [P, SBBp * 2 * P], f8, kind="ExternalInput").ap()
    d_waft8 = nc.dram_tensor("waft8", [P, SBA8 * 2 * P], f8, kind="ExternalInput").ap()
    d_waft16 = nc.dram_tensor("waft16", [P, SBA16 * P], bf16, kind="ExternalInput").ap()
    d_scal = nc.dram_tensor("scal", [N_SLOTS * 4], f32, kind="ExternalInput").ap()
    d_out = nc.dram_tensor("yout", [len(CHUNKS), P, WW], bf16, kind="ExternalOutput").ap()

    def pair_rhs(tile, col_off, dup_off):
        base = tile[:, 0:1]
        return bass.AP(tensor=base.tensor, offset=base.offset + col_off,
                       ap=[[base.ap[0][0], P], [dup_off, 2], [1, WW]])

    def pair_lhs(tile, p):
        s = tile[:, p * 2 * P:(p + 1) * 2 * P]
        return bass.AP(tensor=s.tensor, offset=s.offset,
                       ap=[[s.ap[0][0], P], [P, 2], [1, P]])

    with TileContext(nc) as tc:
        with (
            tc.tile_pool(name="singles", bufs=1) as singles,
            tc.tile_pool(name="work", bufs=4) as work,
            tc.tile_pool(name="ps", bufs=1, space="PSUM") as ps,
        ):
            wme_t = singles.tile([P, BME * P], bf16)
            nc.sync.dma_start(out=wme_t, in_=d_wme)
            # xf split fine-grained: ME group g only needs batches 2g, 2g+1
            xf_t = singles.tile([P, B * XSEC], bf16)
            xf_r = xf_t.rearrange("k (b c) -> k b c", b=B)
            d_xf_r = d_xf.rearrange("b k c -> k b c")
            nc.sync.dma_start(out=xf_r[:, 0:2], in_=d_xf_r[:, 0:2])
            nc.sync.dma_start(out=xf_r[:, 2:4], in_=d_xf_r[:, 2:4])
            scal_t = singles.tile([P, N_SLOTS * 4], f32)
            nc.sync.dma_start(
                out=scal_t,
                in_=bass.AP(tensor=d_scal.tensor, offset=d_scal.offset,
                            ap=[[0, P], [1, N_SLOTS * 4]]),
            )
            wl_t, wb_t, wa_t = {}, {}, {}
            ol, ob = ([0] * (N_SLOTS + 1) for _ in range(2))
            for s in range(N_SLOTS):
                ol[s + 1] = ol[s] + BL[s]
                ob[s + 1] = ob[s] + BBp[s]
            oa8, oa16 = {}, {}
            n8 = n16 = 0
            for s in range(N_SLOTS):
                if s in AFT_FP8_SLOTS:
                    oa8[s] = n8
                    n8 += BA[s]
                else:
                    oa16[s] = n16
                    n16 += BA[s]
            # weights in first-use order; AFT weights interleaved one slot
            # behind (first used AFT_LAG iterations later)
            def dma_wl_wb(s):
                t = singles.tile([P, BL[s] * P], bf16, tag=f"wl{s}")
                nc.sync.dma_start(out=t, in_=d_wlin[:, ol[s] * P:ol[s + 1] * P])
                wl_t[s] = t
                t = singles.tile([P, BBp[s] * 2 * P], f8, tag=f"wb{s}")
                nc.sync.dma_start(out=t, in_=d_wbef[:, ob[s] * 2 * P:ob[s + 1] * 2 * P])
                wb_t[s] = t

            def dma_wa(s):
                if s in AFT_FP8_SLOTS:
                    t = singles.tile([P, BA[s] * 2 * P], f8, tag=f"wa{s}")
                    o = oa8[s]
                    nc.sync.dma_start(out=t, in_=d_waft8[:, o * 2 * P:(o + BA[s]) * 2 * P])
                else:
                    t = singles.tile([P, BA[s] * P], bf16, tag=f"wa{s}")
                    o = oa16[s]
                    nc.sync.dma_start(out=t, in_=d_waft16[:, o * P:(o + BA[s]) * P])
                wa_t[s] = t

            dma_wl_wb(CHUNK_SLOTS[0])
            dma_wl_wb(CHUNK_SLOTS[1])
            nc.sync.dma_start(out=xf_r[:, 4:8], in_=d_xf_r[:, 4:8])
            for i in range(2, N_SLOTS):
                dma_wa(CHUNK_SLOTS[i - 2])
                dma_wl_wb(CHUNK_SLOTS[i])
            dma_wa(CHUNK_SLOTS[N_SLOTS - 2])
            dma_wa(CHUNK_SLOTS[N_SLOTS - 1])

            O8 = (B * SEC + 7) // 4 * 4
            OW = (2 * SEC + 3) // 4 * 4
            xme16 = singles.tile([P, B * SEC], bf16)
            xme8 = singles.tile([P, O8 + B * SEC + 4], f8)
            nc.vector.memset(xme16, 0.0)
            nc.gpsimd.memset(xme8, 0.0)

            def emit_me(g):
                q = 2 * g
                mp = ps.tile([P, XW], f32, tag="me")
                for d in range(BME):
                    ws = q * XSEC + PADX - d
                    nc.tensor.matmul(mp, wme_t[:, d * P:(d + 1) * P],
                                     xf_t[:, ws:ws + XW],
                                     start=(d == 0), stop=(d == BME - 1))
                nc.vector.tensor_copy(out=xme16[:, q * SEC + PADS:(q + 1) * SEC],
                                      in_=mp[:, 0:R])
                nc.vector.tensor_copy(out=xme16[:, (q + 1) * SEC + PADS:(q + 2) * SEC],
                                      in_=mp[:, XSEC:XSEC + R])
                for qq in (q, q + 1):
                    src = xme16[:, qq * SEC + PADS:(qq + 1) * SEC]
                    nc.gpsimd.tensor_copy(out=xme8[:, qq * SEC + PADS:(qq + 1) * SEC],
                                          in_=src)
                    nc.gpsimd.tensor_copy(
                        out=xme8[:, O8 + qq * SEC + PADS + 1:O8 + (qq + 1) * SEC + 1],
                        in_=src)

            emit_me(0)
            emit_me(1)

            NC = len(CHUNKS)
            o_psd, v_psd, w_td = {}, {}, {}

            def emit_lin(j):
                s, h, ci = CHUNKS[j]
                q = 4 * h + 2 * ci
                o_ps = ps.tile([P, WW], f32, tag=f"o{j % 4}")
                o_psd[j] = o_ps
                for d in range(BL[s]):
                    ws = q * SEC + PADS - d
                    nc.tensor.matmul(o_ps, wl_t[s][:, d * P:(d + 1) * P],
                                     xme16[:, ws:ws + WW],
                                     start=(d == 0), stop=False)

            def emit_bef_pointwise(j):
                s, h, ci = CHUNKS[j]
                q = 4 * h + 2 * ci
                a_ap = scal_t[:, 4 * s + 0:4 * s + 1]
                b4_ap = scal_t[:, 4 * s + 1:4 * s + 2]
                k_ap = scal_t.bitcast(i32)[:, 4 * s + 2:4 * s + 3]
                v_ps = ps.tile([P, WW], f32, tag=f"v{j % 3}")
                v_psd[j] = v_ps
                for p in range(BBp[s]):
                    nc.tensor.matmul(v_ps, pair_lhs(wb_t[s], p),
                                     pair_rhs(xme8, q * SEC + PADS - 2 * p, O8),
                                     start=(p == 0), stop=(p == BBp[s] - 1),
                                     perf_mode=DR)
                c_t = work.tile([P, WW], f32, tag="c")
                if C_MODE[j] == "sqrt":
                    u_t = work.tile([P, WW], f32, tag="u")
                    nc.scalar.activation(u_t, v_ps, AF.Abs)
                    nc.scalar.activation(c_t, u_t, AF.Sqrt, scale=b4_ap)
                    nc.scalar.sqrt(c_t, c_t)
                else:
                    t1 = work.tile([P, WW], i32, tag="u")
                    nc.vector.tensor_scalar(
                        out=t1, in0=v_ps.bitcast(i32),
                        scalar1=0x7FFFFFFF, op0=ALU.bitwise_and,
                        scalar2=2, op1=ALU.logical_shift_right)
                    nc.scalar.activation(c_t.bitcast(i32), t1, AF.Identity,
                                         bias=k_ap)
                m_t = work.tile([P, WW], f32, tag="m")
                nc.vector.scalar_tensor_tensor(
                    out=m_t, in0=v_ps, scalar=a_ap, in1=c_t,
                    op0=ALU.mult, op1=ALU.min,
                )
                if s in AFT_FP8_SLOTS:
                    # fp8 w with a one-column-shifted second copy at OW for
                    # the DoubleRow k-tile stride
                    w_t = work.tile([P, OW + 2 * SEC + 4], f8, tag="w8")
                    w_td[j] = w_t
                    nc.vector.scalar_tensor_tensor(
                        out=w_t[:, PADS:2 * SEC], in0=c_t, scalar=-1.0, in1=m_t,
                        op0=ALU.mult, op1=ALU.max,
                    )
                    nc.gpsimd.memset(w_t[:, 0:PADS], 0.0)
                    nc.gpsimd.memset(w_t[:, SEC:SEC + PADS], 0.0)
                    nc.gpsimd.memset(w_t[:, OW:OW + 1], 0.0)
                    nc.gpsimd.tensor_copy(out=w_t[:, OW + 1:OW + 1 + 2 * SEC],
                                          in_=w_t[:, 0:2 * SEC])
                else:
                    w_t = work.tile([P, 2 * SEC], bf16, tag="w")
                    w_td[j] = w_t
                    nc.vector.scalar_tensor_tensor(
                        out=w_t[:, PADS:2 * SEC], in0=c_t, scalar=-1.0, in1=m_t,
                        op0=ALU.mult, op1=ALU.max,
                    )
                    nc.gpsimd.memset(w_t[:, 0:PADS], 0.0)
                    nc.gpsimd.memset(w_t[:, SEC:SEC + PADS], 0.0)

            def emit_aft(j):
                s, _, _ = CHUNKS[j]
                o_ps, w_t = o_psd.pop(j), w_td.pop(j)
                if s in AFT_FP8_SLOTS:
                    for p in range(BA[s]):
                        nc.tensor.matmul(o_ps, pair_lhs(wa_t[s], p),
                                         pair_rhs(w_t, PADS - 2 * p, OW),
                                         start=False, stop=(p == BA[s] - 1),
                                         perf_mode=DR)
                else:
                    for d in range(BA[s]):
                        ws = PADS - d
                        nc.tensor.matmul(o_ps, wa_t[s][:, d * P:(d + 1) * P],
                                         w_t[:, ws:ws + WW],
                                         start=False, stop=(d == BA[s] - 1))
                out_t = work.tile([P, WW], bf16, tag="out")
                if COPY_ENG[j] == "scalar":
                    nc.scalar.activation(out_t, o_ps, AF.Copy)
                else:
                    nc.vector.tensor_copy(out=out_t, in_=o_ps)
                nc.sync.dma_start(out=d_out[j], in_=out_t)

            for t in range(NC + AFT_LAG):
                if t == 2:
                    emit_me(2)
                if t == 4:
                    emit_me(3)
                if t < NC:
                    emit_lin(t)
                if 0 <= t - BEF_LAG < NC:
                    emit_bef_pointwise(t - BEF_LAG)
                if 0 <= t - AFT_LAG < NC:
                    emit_aft(t - AFT_LAG)
    nc.compile()
    return nc


def _prep(inputs):
    key = "prog"
    if key not in _CACHE:
        meta = _build_host(
            inputs["me_fir"], inputs["lin_fir"], inputs["nlin_fir_before"],
            inputs["nlin_fir_after"], inputs["lpf_lin_b"], inputs["lpf_lin_a"],
            inputs["lpf_nlin_b"], inputs["lpf_nlin_a"],
            np.asarray(inputs["lin_gain"], np.float64),
            np.asarray(inputs["nlin_a"], np.float64),
            np.asarray(inputs["nlin_b"], np.float64),
        )
        _CACHE[key] = (meta, _build_program(meta))
    return _CACHE[key]


def _in_maps(meta, x):
    xf = _fold_x(np.asarray(x, np.float32))
    return [
        {"xf": xf, "wme": meta["wme"], "wlin": meta["wlin"][c],
         "wbef": meta["wbef"][c], "waft8": meta["waft8"][c],
         "waft16": meta["waft16"][c], "scal": meta["scal"][c]}
        for c in range(N_CORES)
    ]


def _decode(meta, youts):
    PADS = meta["PADS"]
    SEC = PADS + R
    slot_ch = meta["slot_ch"]
    out = np.zeros((B, F, T), np.float32)
    for c in range(N_CORES):
        yo = np.asarray(youts[c], dtype=np.float32)
        for j, (s, h, ci) in enumerate(CHUNKS):
            if s == 6 and c >= 2:
                continue
            f = slot_ch[c, s]
            inv = 1.0 / meta["gam"][c, s]
            for bi, off in ((0, 0), (1, SEC)):
                b = 4 * h + 2 * ci + bi
                out[b, f, :] = yo[j, :, off:off + R].T.reshape(R * P)[:T] * inv
    return out


def kernel(**inputs):
    meta, nc = _prep(inputs)
    from concourse.bass_utils import run_bass_kernel_spmd

    res = run_bass_kernel_spmd(nc, _in_maps(meta, inputs["x"]),
                               core_ids=list(range(N_CORES)),
                               trace=bool(inputs.get("_trace", False)))
    out = _decode(meta, [res.results[c]["yout"] for c in range(N_CORES)])
    if inputs.get("_return_res", False):
        return out, res
    return out


# revision 3
# speedup vs baseline: 1.0212x; 1.0212x over previous
"""DRNL filterbank Trainium2 kernel, v3.

Banded-Toeplitz formulation (see kernel.py). Engine-balanced pipeline:
  - ME + LIN FIRs: bf16 matmuls.
  - BEF FIR: fp8e4m3 DoubleRow band pairs (k-tile 1 reads a one-column-
    shifted second copy of the fp8 signal at a large stride — the PE
    rejects small k-tile strides).
  - AFT FIR: bf16 single-band matmuls on the bf16 broken-stick output
    (pairing AFT would need a per-chunk shifted copy of w, which choked
    the slow gpsimd engine in v2).
  - Broken-stick: w = clip(a*v, +-b|v|^0.25). The threshold c=b|v|^0.25
    is computed two ways, alternated per chunk to balance engines:
      "sqrt": c = sqrt(sqrt(b^4|v|))          (3 scalar-engine acts)
      "int":  c ~= bitcast((bits(v)&0x7fffffff)>>2 + K_f)  (1 vector
              tensor_scalar + 1 scalar Identity-with-int-bias act);
              K_f host-tuned per channel, ~3% error on c — harmless,
              the NL path is ~500x below the linear path.
  - Output copies (psum -> bf16) alternate scalar/vector per chunk.
  - lag-2 software pipeline; heavy/light slot interleave; LIN issued
    before BEF so the PE fills while fp8 signal copies are produced.
Sharding: channels across 8 cores, slot-structured SPMD (as v1).
"""
import numpy as np
import ml_dtypes

P = 128
B, T, F = 8, 20000, 50
R = (T + P - 1) // P
N_CORES = 8
N_SLOTS = 7
TRUNC_LIN = 6e-4
TRUNC_NL = 2.5e-2
# slots whose AFT conv runs as fp8 DoubleRow band pairs (their channels sit
# well below the global absmax, and the whole channel is pre-scaled by a
# power of two that the host decode divides back out)
AFT_FP8_SLOTS = ()
IR_LEN = 4096
BME = 5
PADX = 4

_CACHE = {}

BF16 = ml_dtypes.bfloat16
FP8 = ml_dtypes.float8_e4m3fn

# chunk emission order: interleave heavy and light slots. First group leads
# with a tiny slot (cheap pipeline fill), last group ends on the lightest
# (short drain).
CHUNK_SLOTS = [4, 0, 6, 1, 5, 2, 3]
_GROUP_ORDERS = {
    (0, 0): [4, 0, 6, 1, 5, 2, 3],
    (0, 1): [0, 6, 1, 5, 2, 4, 3],
    (1, 0): [0, 6, 1, 5, 2, 4, 3],
    (1, 1): [0, 6, 1, 5, 2, 3, 4],
}
CHUNKS = [(s, h, ci) for (h, ci) in [(0, 0), (0, 1), (1, 0), (1, 1)]
          for s in _GROUP_ORDERS[(h, ci)]]
# per-chunk c-computation mode: 18 "int" / 10 "sqrt" balances scalar vs vector
C_MODE = ["int" if (j * 18) % 28 < 18 else "sqrt" for j in range(len(CHUNKS))]
# psum->bf16 output copy engine per chunk
COPY_ENG = ["scalar"] * len(CHUNKS)
BEF_LAG = 1   # BEF_j issues after LIN_{j+BEF_LAG}
AFT_LAG = 3   # AFT_j issues in iteration j+AFT_LAG


def _lfilter_vec(x, b, a):
    b0, b1, b2 = b[:, 0], b[:, 1], b[:, 2]
    a1, a2 = a[:, 1], a[:, 2]
    y = np.zeros_like(x)
    z1 = np.zeros(x.shape[0])
    z2 = np.zeros(x.shape[0])
    for t in range(x.shape[-1]):
        xt = x[:, t]
        yt = b0 * xt + z1
        z1 = b1 * xt - a1 * yt + z2
        z2 = b2 * xt - a2 * yt
        y[:, t] = yt
    return y


def _cascade_ir(b, a, n, times):
    h = np.zeros((b.shape[0], n))
    h[:, 0] = 1.0
    for _ in range(times):
        h = _lfilter_vec(h, b, a)
    return h


def _trunc(h, tol):
    m = np.abs(h).max()
    idx = np.nonzero(np.abs(h) > tol * m)[0]
    return h[: int(idx[-1]) + 1] if len(idx) else h[:1]


def _nb(h):
    return (len(h) + P - 2) // P + 1


def _toeplitz_band(h, d):
    k = np.arange(P)[:, None]
    i = np.arange(P)[None, :]
    idx = P * d + i - k
    ok = (idx >= 0) & (idx < len(h))
    return np.where(ok, np.asarray(h, np.float64)[np.clip(idx, 0, len(h) - 1)], 0.0)


def _bands_cat(h, nb, dtype):
    W = np.concatenate([_toeplitz_band(h, d) for d in range(nb)], axis=1)
    return np.ascontiguousarray(W).astype(dtype)


def _pairs_cat(h, nbp, dtype):
    mats = []
    for p in range(nbp):
        mats.append(_toeplitz_band(h, 2 * p))
        mats.append(_toeplitz_band(h, 2 * p + 1))
    W = np.concatenate(mats, axis=1)
    return np.ascontiguousarray(W).astype(dtype)


def _tune_k(b):
    """Best int32 bias K: bitcast((bits(x)>>2)+K) ~= b*x**0.25 over x range."""
    x = np.float32(np.logspace(-6, 1.5, 4000))
    t1 = (x.view(np.int32) & 0x7FFFFFFF) >> 2
    ref = np.float64(b) * np.float64(x) ** 0.25
    b0 = 127 << 23
    base = np.int64(np.float32(b).view(np.int32)) - b0 // 4
    best = (1e9, 0)
    for sig in np.linspace(-0.02, 0.10, 121):
        K = np.int64(base - int(sig * (1 << 23)))
        c = (t1.astype(np.int64) + K).astype(np.int32).view(np.float32)
        e = np.abs(c - ref) / ref
        best = min(best, (float(e.max()), int(K)))
    return best[1]


def _build_host(me_fir, lin_fir, nlin_fir_before, nlin_fir_after,
                lpf_lin_b, lpf_lin_a, lpf_nlin_b, lpf_nlin_a,
                lin_gain, nlin_a, nlin_b):
    ir4 = _cascade_ir(lpf_lin_b.astype(np.float64), lpf_lin_a.astype(np.float64), IR_LEN, 4)
    ir3 = _cascade_ir(lpf_nlin_b.astype(np.float64), lpf_nlin_a.astype(np.float64), IR_LEN, 3)

    scale = 10.0 ** ((93.98 - 100.0) / 20.0)
    ME = np.asarray(me_fir, np.float64) * scale

    LIN, BEF, AFT = [], [], []
    for f in range(F):
        LIN.append(_trunc(lin_gain[f] * np.convolve(np.asarray(lin_fir[f], np.float64), ir4[f]), TRUNC_LIN))
        BEF.append(_trunc(np.asarray(nlin_fir_before[f], np.float64), TRUNC_NL))
        AFT.append(_trunc(np.convolve(np.asarray(nlin_fir_after[f], np.float64), ir3[f]), TRUNC_NL))

    nbp = lambda h: (_nb(h) + 1) // 2
    cost = [_nb(LIN[f]) + nbp(BEF[f]) + _nb(AFT[f]) for f in range(F)]
    order = np.argsort(-np.asarray(cost), kind="stable")

    slot_ch = np.zeros((N_CORES, N_SLOTS), np.int64)
    for s in range(6):
        for c in range(N_CORES):
            slot_ch[c, s] = order[8 * s + c]
    for c in range(N_CORES):
        slot_ch[c, 6] = order[48 + (c % 2)]

    BL = [max(_nb(LIN[slot_ch[c, s]]) for c in range(N_CORES)) for s in range(N_SLOTS)]
    BBp = [max(nbp(BEF[slot_ch[c, s]]) for c in range(N_CORES)) for s in range(N_SLOTS)]
    # AFT band count: pairs for fp8 slots, single bands otherwise
    BA = []
    for s in range(N_SLOTS):
        if s in AFT_FP8_SLOTS:
            BA.append(max(nbp(AFT[slot_ch[c, s]]) for c in range(N_CORES)))
        else:
            BA.append(max(_nb(AFT[slot_ch[c, s]]) for c in range(N_CORES)))

    sh = max(max(BL) - 1, 2 * max(BBp) - 1,
             max((2 * BA[s] - 1) if s in AFT_FP8_SLOTS else (BA[s] - 1)
                 for s in range(N_SLOTS)))
    PADS = (sh + 1 + 1) // 2 * 2

    # per-channel power-of-two output scale: lifts fp8 AFT taps into the
    # representable range; the decode divides it back out
    gam = np.ones((N_CORES, N_SLOTS), np.float64)
    for c in range(N_CORES):
        for s in AFT_FP8_SLOTS:
            f = slot_ch[c, s]
            gam[c, s] = 2.0 ** np.floor(np.log2(64.0 / np.abs(AFT[f]).max()))

    wme = _bands_cat(ME, BME, BF16)
    wlin = [np.concatenate([_bands_cat(gam[c, s] * LIN[slot_ch[c, s]], BL[s], BF16)
                            for s in range(N_SLOTS)], axis=1)
            for c in range(N_CORES)]
    wbef = [np.concatenate([_pairs_cat(BEF[slot_ch[c, s]], BBp[s], FP8) for s in range(N_SLOTS)], axis=1)
            for c in range(N_CORES)]
    waft8, waft16 = [], []
    for c in range(N_CORES):
        p8 = [_pairs_cat(gam[c, s] * AFT[slot_ch[c, s]], BA[s], FP8)
              for s in range(N_SLOTS) if s in AFT_FP8_SLOTS]
        p16 = [_bands_cat(gam[c, s] * AFT[slot_ch[c, s]], BA[s], BF16)
               for s in range(N_SLOTS) if s not in AFT_FP8_SLOTS]
        waft8.append(np.concatenate(p8, axis=1) if p8 else np.zeros((P, 2 * P), FP8))
        waft16.append(np.concatenate(p16, axis=1) if p16 else np.zeros((P, P), BF16))

    scal = np.zeros((N_CORES, N_SLOTS * 4), np.float32)
    for c in range(N_CORES):
        for s in range(N_SLOTS):
            f = slot_ch[c, s]
            scal[c, 4 * s + 0] = nlin_a[f]
            scal[c, 4 * s + 1] = float(nlin_b[f]) ** 4
            scal[c].view(np.int32)[4 * s + 2] = _tune_k(float(nlin_b[f]))

    return {
        "slot_ch": slot_ch, "BL": BL, "BBp": BBp, "BA": BA, "PADS": PADS,
        "gam": gam,
        "wme": wme, "wlin": wlin, "wbef": wbef,
        "waft8": waft8, "waft16": waft16, "scal": scal,
    }


def _fold_x(x):
    xp = np.zeros((B, R * P), np.float32)
    xp[:, :T] = x
    xf = np.zeros((B, P, PADX + R), np.float32)
    xf[:, :, PADX:] = xp.reshape(B, R, P).transpose(0, 2, 1)
    return xf.astype(BF16)


def _build_program(meta):
    import concourse.bacc as bacc
    import concourse.bass as bass
    from concourse import mybir
    from concourse.tile import TileContext

    BL, BBp, BA, PADS = meta["BL"], meta["BBp"], meta["BA"], meta["PADS"]
    SBL, SBBp = sum(BL), sum(BBp)
    SBA8 = max(sum(BA[s] for s in range(N_SLOTS) if s in AFT_FP8_SLOTS), 1)
    SBA16 = max(sum(BA[s] for s in range(N_SLOTS) if s not in AFT_FP8_SLOTS), 1)
    SEC = PADS + R
    WW = SEC + R
    XSEC = PADX + R
    XW = XSEC + R
    f32, bf16, f8 = mybir.dt.float32, mybir.dt.bfloat16, mybir.dt.float8e4
    i32 = mybir.dt.int32
    AF = mybir.ActivationFunctionType
    ALU = mybir.AluOpType
    DR = mybir.MatmulPerfMode.DoubleRow

    nc = bacc.Bacc("TRN2", target_bir_lowering=False, debug=False, num_devices=N_CORES)
    d_xf = nc.dram_tensor("xf", [B, P, XSEC], bf16, kind="ExternalInput").ap()
    d_wme = nc.dram_tensor("wme", [P, BME * P], bf16, kind="ExternalInput").ap()
    d_wlin = nc.dram_tensor("wlin", [P, SBL * P], bf16, kind="ExternalInput").ap()
    d_wbef = nc.dram_tensor("wbef", [P, SBBp * 2 * P], f8, kind="ExternalInput").ap()
    d_waft8 = nc.dram_tensor("waft8", [P, SBA8 * 2 * P], f8, kind="ExternalInput").ap()
    d_waft16 = nc.dram_tensor("waft16", [P, SBA16 * P], bf16, kind="ExternalInput").ap()
    d_scal = nc.dram_tensor("scal", [N_SLOTS * 4], f32, kind="ExternalInput").ap()
    d_out = nc.dram_tensor("yout", [len(CHUNKS), P, WW], bf16, kind="ExternalOutput").ap()

    def pair_rhs(tile, col_off, dup_off):
        base = tile[:, 0:1]
        return bass.AP(tensor=base.tensor, offset=base.offset + col_off,
                       ap=[[base.ap[0][0], P], [dup_off, 2], [1, WW]])

    def pair_lhs(tile, p):
        s = tile[:, p * 2 * P:(p + 1) * 2 * P]
        return bass.AP(tensor=s.tensor, offset=s.offset,
                       ap=[[s.ap[0][0], P], [P, 2], [1, P]])

    with TileContext(nc) as tc:
        with (
            tc.tile_pool(name="singles", bufs=1) as singles,
            tc.tile_pool(name="work", bufs=4) as work,
            tc.tile_pool(name="ps", bufs=1, space="PSUM") as ps,
        ):
            wme_t = singles.tile([P, BME * P], bf16)
            nc.sync.dma_start(out=wme_t, in_=d_wme)
            # xf split fine-grained: ME group g only needs batches 2g, 2g+1
            xf_t = singles.tile([P, B * XSEC], bf16)
            xf_r = xf_t.rearrange("k (b c) -> k b c", b=B)
            d_xf_r = d_xf.rearrange("b k c -> k b c")
            nc.sync.dma_start(out=xf_r[:, 0:2], in_=d_xf_r[:, 0:2])
            nc.sync.dma_start(out=xf_r[:, 2:4], in_=d_xf_r[:, 2:4])
            scal_t = singles.tile([P, N_SLOTS * 4], f32)
            nc.sync.dma_start(
                out=scal_t,
                in_=bass.AP(tensor=d_scal.tensor, offset=d_scal.offset,
                            ap=[[0, P], [1, N_SLOTS * 4]]),
            )
            wl_t, wb_t, wa_t = {}, {}, {}
            ol, ob = ([0] * (N_SLOTS + 1) for _ in range(2))
            for s in range(N_SLOTS):
                ol[s + 1] = ol[s] + BL[s]
                ob[s + 1] = ob[s] + BBp[s]
            oa8, oa16 = {}, {}
            n8 = n16 = 0
            for s in range(N_SLOTS):
                if s in AFT_FP8_SLOTS:
                    oa8[s] = n8
                    n8 += BA[s]
                else:
                    oa16[s] = n16
                    n16 += BA[s]
            # weights in first-use order; AFT weights interleaved one slot
            # behind (first used AFT_LAG iterations later)
            def dma_wl_wb(s):
                t = singles.tile([P, BL[s] * P], bf16, tag=f"wl{s}")
                nc.sync.dma_start(out=t, in_=d_wlin[:, ol[s] * P:ol[s + 1] * P])
                wl_t[s] = t
                t = singles.tile([P, BBp[s] * 2 * P], f8, tag=f"wb{s}")
                nc.sync.dma_start(out=t, in_=d_wbef[:, ob[s] * 2 * P:ob[s + 1] * 2 * P])
                wb_t[s] = t

            def dma_wa(s):
                if s in AFT_FP8_SLOTS:
                    t = singles.tile([P, BA[s] * 2 * P], f8, tag=f"wa{s}")
                    o = oa8[s]
                    nc.sync.dma_start(out=t, in_=d_waft8[:, o * 2 * P:(o + BA[s]) * 2 * P])
                else:
                    t = singles.tile([P, BA[s] * P], bf16, tag=f"wa{s}")
                    o = oa16[s]
                    nc.sync.dma_start(out=t, in_=d_waft16[:, o * P:(o + BA[s]) * P])
                wa_t[s] = t

            dma_wl_wb(CHUNK_SLOTS[0])
            dma_wl_wb(CHUNK_SLOTS[1])
            nc.sync.dma_start(out=xf_r[:, 4:8], in_=d_xf_r[:, 4:8])
            for i in range(2, N_SLOTS):
                dma_wa(CHUNK_SLOTS[i - 2])
                dma_wl_wb(CHUNK_SLOTS[i])
            dma_wa(CHUNK_SLOTS[N_SLOTS - 2])
            dma_wa(CHUNK_SLOTS[N_SLOTS - 1])

            O8 = (B * SEC + 7) // 4 * 4
            OW = (2 * SEC + 3) // 4 * 4
            xme16 = singles.tile([P, B * SEC], bf16)
            xme8 = singles.tile([P, O8 + B * SEC + 4], f8)
            nc.vector.memset(xme16, 0.0)
            nc.gpsimd.memset(xme8, 0.0)

            def emit_me(g):
                q = 2 * g
                mp = ps.tile([P, XW], f32, tag="me")
                for d in range(BME):
                    ws = q * XSEC + PADX - d
                    nc.tensor.matmul(mp, wme_t[:, d * P:(d + 1) * P],
                                     xf_t[:, ws:ws + XW],
                                     start=(d == 0), stop=(d == BME - 1))
                nc.vector.tensor_copy(out=xme16[:, q * SEC + PADS:(q + 1) * SEC],
                                      in_=mp[:, 0:R])
                nc.vector.tensor_copy(out=xme16[:, (q + 1) * SEC + PADS:(q + 2) * SEC],
                                      in_=mp[:, XSEC:XSEC + R])
                for qq in (q, q + 1):
                    src = xme16[:, qq * SEC + PADS:(qq + 1) * SEC]
                    nc.gpsimd.tensor_copy(out=xme8[:, qq * SEC + PADS:(qq + 1) * SEC],
                                          in_=src)
                    nc.gpsimd.tensor_copy(
                        out=xme8[:, O8 + qq * SEC + PADS + 1:O8 + (qq + 1) * SEC + 1],
                        in_=src)

            emit_me(0)
            emit_me(1)

            NC = len(CHUNKS)
            o_psd, v_psd, w_td = {}, {}, {}
            w_bufs = []
            for i in range(AFT_LAG + 1):
                wbuf = singles.tile([P, 2 * SEC], bf16, tag=f"wbuf{i}")
                nc.vector.memset(wbuf[:, 0:PADS], 0.0)
                w_bufs.append(wbuf)

            def emit_lin(j):
                s, h, ci = CHUNKS[j]
                q = 4 * h + 2 * ci
                o_ps = ps.tile([P, WW], f32, tag=f"o{j % 4}")
                o_psd[j] = o_ps
                for d in range(BL[s]):
                    ws = q * SEC + PADS - d
                    nc.tensor.matmul(o_ps, wl_t[s][:, d * P:(d + 1) * P],
                                     xme16[:, ws:ws + WW],
                                     start=(d == 0), stop=False)

            def emit_bef_pointwise(j):
                s, h, ci = CHUNKS[j]
                q = 4 * h + 2 * ci
                a_ap = scal_t[:, 4 * s + 0:4 * s + 1]
                b4_ap = scal_t[:, 4 * s + 1:4 * s + 2]
                k_ap = scal_t.bitcast(i32)[:, 4 * s + 2:4 * s + 3]
                v_ps = ps.tile([P, WW], f32, tag=f"v{j % 3}")
                v_psd[j] = v_ps
                for p in range(BBp[s]):
                    nc.tensor.matmul(v_ps, pair_lhs(wb_t[s], p),
                                     pair_rhs(xme8, q * SEC + PADS - 2 * p, O8),
                                     start=(p == 0), stop=(p == BBp[s] - 1),
                                     perf_mode=DR)
                c_t = work.tile([P, WW], f32, tag="c")
                if C_MODE[j] == "sqrt":
                    u_t = work.tile([P, WW], f32, tag="u")
                    nc.scalar.activation(u_t, v_ps, AF.Abs)
                    nc.scalar.activation(c_t, u_t, AF.Sqrt, scale=b4_ap)
                    nc.scalar.sqrt(c_t, c_t)
                else:
                    t1 = work.tile([P, WW], i32, tag="u")
                    nc.vector.tensor_scalar(
                        out=t1, in0=v_ps.bitcast(i32),
                        scalar1=0x7FFFFFFF, op0=ALU.bitwise_and,
                        scalar2=2, op1=ALU.logical_shift_right)
                    nc.scalar.activation(c_t.bitcast(i32), t1, AF.Identity,
                                         bias=k_ap)
                m_t = work.tile([P, WW], f32, tag="m")
                nc.vector.scalar_tensor_tensor(
                    out=m_t, in0=v_ps, scalar=a_ap, in1=c_t,
                    op0=ALU.mult, op1=ALU.min,
                )
                # w buffers are static: lead pads were zeroed once at startup
                # and nothing ever writes them. The stt covers [PADS, 2*SEC);
                # only the mid pad needs re-zeroing, on the same queue as the
                # stt so the AFT wait does not hop engines.
                w_t = w_bufs[j % len(w_bufs)]
                w_td[j] = w_t
                nc.vector.scalar_tensor_tensor(
                    out=w_t[:, PADS:2 * SEC], in0=c_t, scalar=-1.0, in1=m_t,
                    op0=ALU.mult, op1=ALU.max,
                )
                nc.vector.memset(w_t[:, SEC:SEC + PADS], 0.0)

            def emit_aft(j):
                s, _, _ = CHUNKS[j]
                o_ps, w_t = o_psd.pop(j), w_td.pop(j)
                if s in AFT_FP8_SLOTS:
                    for p in range(BA[s]):
                        nc.tensor.matmul(o_ps, pair_lhs(wa_t[s], p),
                                         pair_rhs(w_t, PADS - 2 * p, OW),
                                         start=False, stop=(p == BA[s] - 1),
                                         perf_mode=DR)
                else:
                    for d in range(BA[s]):
                        ws = PADS - d
                        nc.tensor.matmul(o_ps, wa_t[s][:, d * P:(d + 1) * P],
                                         w_t[:, ws:ws + WW],
                                         start=False, stop=(d == BA[s] - 1))
                out_t = work.tile([P, WW], bf16, tag="out")
                if COPY_ENG[j] == "scalar":
                    nc.scalar.activation(out_t, o_ps, AF.Copy)
                else:
                    nc.vector.tensor_copy(out=out_t, in_=o_ps)
                nc.sync.dma_start(out=d_out[j], in_=out_t)

            for t in range(NC + AFT_LAG):
                if t == 2:
                    emit_me(2)
                if t == 4:
                    emit_me(3)
                if t < NC:
                    emit_lin(t)
                if 0 <= t - BEF_LAG < NC:
                    emit_bef_pointwise(t - BEF_LAG)
                if 0 <= t - AFT_LAG < NC:
                    emit_aft(t - AFT_LAG)
    nc.compile()
    return nc


def _prep(inputs):
    key = "prog"
    if key not in _CACHE:
        meta = _build_host(
            inputs["me_fir"], inputs["lin_fir"], inputs["nlin_fir_before"],
            inputs["nlin_fir_after"], inputs["lpf_lin_b"], inputs["lpf_lin_a"],
            inputs["lpf_nlin_b"], inputs["lpf_nlin_a"],
            np.asarray(inputs["lin_gain"], np.float64),
            np.asarray(inputs["nlin_a"], np.float64),
            np.asarray(inputs["nlin_b"], np.float64),
        )
        _CACHE[key] = (meta, _build_program(meta))
    return _CACHE[key]


def _in_maps(meta, x):
    xf = _fold_x(np.asarray(x, np.float32))
    return [
        {"xf": xf, "wme": meta["wme"], "wlin": meta["wlin"][c],
         "wbef": meta["wbef"][c], "waft8": meta["waft8"][c],
         "waft16": meta["waft16"][c], "scal": meta["scal"][c]}
        for c in range(N_CORES)
    ]


def _decode(meta, youts):
    PADS = meta["PADS"]
    SEC = PADS + R
    slot_ch = meta["slot_ch"]
    out = np.zeros((B, F, T), np.float32)
    for c in range(N_CORES):
        yo = np.asarray(youts[c], dtype=np.float32)
        for j, (s, h, ci) in enumerate(CHUNKS):
            if s == 6 and c >= 2:
                continue
            f = slot_ch[c, s]
            inv = 1.0 / meta["gam"][c, s]
            for bi, off in ((0, 0), (1, SEC)):
                b = 4 * h + 2 * ci + bi
                out[b, f, :] = yo[j, :, off:off + R].T.reshape(R * P)[:T] * inv
    return out


def kernel(**inputs):
    meta, nc = _prep(inputs)
    from concourse.bass_utils import run_bass_kernel_spmd

    res = run_bass_kernel_spmd(nc, _in_maps(meta, inputs["x"]),
                               core_ids=list(range(N_CORES)),
                               trace=bool(inputs.get("_trace", False)))
    out = _decode(meta, [res.results[c]["yout"] for c in range(N_CORES)])
    if inputs.get("_return_res", False):
        return out, res
    return out
